# revision 20
# baseline (speedup 1.0000x reference)
"""LorentzNodeBlock — Trainium2 Bass kernel, 8 NeuronCores, scatter-free layout.

Sharding strategy (differs from the hint's edge-parallel+allreduce: we shard
by DESTINATION node so no collective is needed):

  * Host sorts nodes by in-degree and packs them into blocks of 64 nodes
    (one node per (core, group) lane; 8 cores x 8 groups = 64 lanes). Each
    node's incoming-edge list is zero-padded to the block's max degree, so
    all 64 lanes share one identical static layout -> one SPMD program.
  * Per-core edge payload is [120, T]: 8 groups x 15 rows (q_row, edge_attr^T)
    with edges along the free axis. q_row = minkowski(x)[row] is gathered on
    the host during layout construction (index-driven data staging); all
    model FLOPs on the [E,*] edge data happen on device.
  * Device per core: block-diagonal (8-way packed) first-MLP matmul on PE,
    fused bias+ReLU on the scalar engine, then segmented reduce_sum ops on
    the vector engine with static access patterns (uniform segment length
    per run of equal-degree blocks). The second edge-MLP linear layer is
    pushed through the segment sum (linearity): it is applied to the [N,14]
    node sums, not the [E,14] edge values. Zero-pad slots contribute
    relu(b1a) each; the device subtracts pad_count * relu(b1a) before the
    mean. Node MLP also runs on device in the same 8-group layout.
  * Host inverse-permutes the [112, S] per-core outputs back to [N, 14].
"""

import os
import sys
import types
import numpy as np

N = 100000
E = 6400000
HID = 14
NCORES = 8
NGROUPS = 8
NLANES = NCORES * NGROUPS
CHUNK = 1536          # edge slots per PSUM chunk (free dim, 3 banks)
SUPER = 6144          # edge slots per DMA transfer (4 chunks, 1.5 MB bf16)
MM_F = 512            # matmul moving free dim / PSUM bank (f32)

_MINK = np.array([-1.0, 1.0, 1.0, 1.0], dtype=np.float32)


# ---------------------------------------------------------------------------
# axon NTFF shim: lets run_bass_kernel_spmd(trace=True) work when the image's
# antenv package lacks axon_hooks. Harmless when tracing is off.
# ---------------------------------------------------------------------------
def _install_ntff_shim():
    try:
        from antenv.axon_hooks import get_axon_ntff_profile_hook  # noqa: F401
        have = True
    except ImportError:
        have = False
    if not have:
        try:
            import antenv
        except ImportError:
            return
        mod = types.ModuleType("antenv.axon_hooks")
        _hook = [None]
        mod.set_axon_ntff_profile_hook = lambda h: _hook.__setitem__(0, h)
        mod.get_axon_ntff_profile_hook = lambda: _hook[0]
        sys.modules["antenv.axon_hooks"] = mod
        antenv.axon_hooks = mod
    try:
        from antenv.axon_hooks import (get_axon_ntff_profile_hook,
                                       set_axon_ntff_profile_hook)
        if get_axon_ntff_profile_hook() is None:
            from trn_agent_boot.trn_boot import _ntff_profile_via_ctypes
            set_axon_ntff_profile_hook(
                _ntff_profile_via_ctypes('/opt/axon/libaxon_pjrt.so'))
    except Exception:
        pass


# ---------------------------------------------------------------------------
# Host-side layout
# ---------------------------------------------------------------------------
class Layout:
    __slots__ = ("node_sorted", "blk_deg", "blk_off", "n_blocks", "S", "T",
                 "n_chunks", "reduce_ops", "deg", "cs", "edge_sorted")


def build_layout(col):
    """col: int array [E] of destination nodes. Pure index work."""
    lay = Layout()
    deg = np.bincount(col, minlength=N).astype(np.int64)
    n_pad_nodes = (-N) % 64
    deg_p = np.concatenate([deg, np.zeros(n_pad_nodes, np.int64)])
    node_sorted = np.argsort(deg_p, kind="stable")
    B = deg_p.shape[0] // 64
    node_mat = node_sorted.reshape(B, 64)
    blk_deg = deg_p[node_mat].max(axis=1)          # [B] padded degree
    # pack blocks into CHUNK-aligned slots
    blk_off = np.zeros(B, np.int64)
    off = 0
    for b in range(B):
        d = int(blk_deg[b])
        if (off % CHUNK) + d > CHUNK:
            off = (off // CHUNK + 1) * CHUNK
        blk_off[b] = off
        off += d
    T = ((off + SUPER - 1) // SUPER) * SUPER
    # reduce ops: runs of equal-d blocks within one chunk
    # (chunk_idx, in_off_in_chunk, out_slot, n_blocks, d)
    ops = []
    b = 0
    while b < B:
        d = int(blk_deg[b])
        c0 = int(blk_off[b]) // CHUNK
        nb = 1
        while (b + nb < B and int(blk_deg[b + nb]) == d
               and int(blk_off[b + nb]) // CHUNK == c0
               and int(blk_off[b + nb]) == int(blk_off[b]) + nb * d):
            nb += 1
        if d > 0:
            ops.append((c0, int(blk_off[b]) % CHUNK, b, nb, d))
        b += nb
    cs = np.zeros(N + 1, np.int64)
    cs[1:] = np.cumsum(deg)
    lay.node_sorted = node_sorted
    lay.blk_deg = blk_deg
    lay.blk_off = blk_off
    lay.n_blocks = B
    lay.S = B
    lay.T = int(T)
    lay.n_chunks = int(T) // CHUNK
    lay.reduce_ops = ops
    lay.deg = deg_p
    lay.cs = cs
    lay.edge_sorted = np.argsort(col, kind="stable")
    return lay


def build_src_slots(lay):
    """[64, T] int64 edge-id per lane slot, -1 for padding."""
    B, T = lay.n_blocks, lay.T
    node_mat = lay.node_sorted.reshape(B, 64)
    deg_nm = lay.deg[node_mat]                       # [B, 64]
    D = int(lay.blk_deg.max())
    k = np.arange(D, dtype=np.int64)
    valid = k[None, None, :] < deg_nm[:, :, None]    # [B, 64, D]
    cs_nm = np.where(node_mat < N, lay.cs[np.minimum(node_mat, N - 1)], 0)
    idx = cs_nm[:, :, None] + k[None, None, :]
    idx = np.minimum(idx, E - 1)
    src = np.where(valid, lay.edge_sorted[idx], -1)  # [B, 64, D]
    slot_valid = k[None, :] < lay.blk_deg[:, None]   # [B, D]
    pos = (lay.blk_off[:, None] + k[None, :])        # [B, D]
    out = np.full((64, T), -1, np.int64)
    out[:, pos[slot_valid]] = src.transpose(1, 0, 2)[:, slot_valid]
    return out


def lane_of(c, g):
    return c * 8 + g


def build_core_payload(lay, src_slots, qe, edge_attr, core, dtype=np.float32):
    """[128, T]: rows 16g+0 = q_row, 16g+1+j = edge_attr[:, j],
    16g+15 = valid mask (1.0 real edge / 0.0 pad -> folds b1a into the
    matmul and makes pad slots contribute exactly 0 after relu)."""
    T = lay.T
    pay = np.zeros((NGROUPS * 16, T), dtype)
    for g in range(NGROUPS):
        s = src_slots[lane_of(core, g)]
        m = s >= 0
        sc = np.where(m, s, 0)
        pay[16 * g, :] = np.where(m, qe[sc], 0.0).astype(dtype)
        ea = edge_attr[sc].astype(dtype)
        ea[~m] = 0
        pay[16 * g + 1:16 * g + 15, :] = ea.T
        pay[16 * g + 15, :] = m.astype(dtype)
    return pay


def build_core_aux(lay, x, core):
    """invdeg_rep [112,S], x_t [32,S] (rows k*8+g)."""
    B = lay.n_blocks
    node_mat = lay.node_sorted.reshape(B, 64)
    nodes = node_mat[:, core * 8:(core + 1) * 8].T        # [8, B] (g, b)
    degs = lay.deg[nodes].astype(np.float32)              # [8, B]
    invdeg = 1.0 / np.maximum(degs, 1.0)
    invdeg_rep = np.repeat(invdeg, HID, axis=0)           # [112, B]
    real = nodes < N
    xn = x[np.minimum(nodes, N - 1)].astype(np.float32)   # [8, B, 4]
    xn[~real] = 0
    x_t = xn.transpose(2, 0, 1).reshape(32, B)            # rows k*8+g
    return invdeg_rep, x_t


def build_weights(w1a, b1a, w1b, b1b, w2a, b2a, w2b, b2b, dtype=np.float32):
    def blkdiag(w, nin, nout):
        out = np.zeros((NGROUPS * nin, NGROUPS * nout), np.float32)
        for g in range(NGROUPS):
            out[g * nin:(g + 1) * nin, g * nout:(g + 1) * nout] = w
        return out
    W = {}
    w1x = np.concatenate([w1a, b1a[None, :]], axis=0)      # [16, 14]
    W["W1blk"] = blkdiag(w1x, 16, 14).astype(dtype)        # [128, 112]
    W["b1a_rep"] = np.tile(b1a, NGROUPS).astype(np.float32)[:, None]  # [112,1]
    W["W1Bblk"] = blkdiag(w1b, 14, 14)                     # [112, 112]
    W["b1b_rep"] = np.tile(b1b, NGROUPS).astype(np.float32)[:, None]
    W["W2Ablk"] = blkdiag(w2a[1:], 14, 14)                 # [112, 112]
    w2a0 = np.zeros((NGROUPS, NGROUPS * 14), np.float32)
    for g in range(NGROUPS):
        w2a0[g, g * 14:(g + 1) * 14] = w2a[0]
    W["W2A0blk"] = w2a0                                    # [8, 112]
    W["b2a_rep"] = np.tile(b2a, NGROUPS).astype(np.float32)[:, None]
    W["W2Bblk"] = blkdiag(w2b, 14, 14)
    W["b2b_rep"] = np.tile(b2b, NGROUPS).astype(np.float32)[:, None]
    return W


# ---------------------------------------------------------------------------
# Numpy emulation of the device program (for layout/logic validation)
# ---------------------------------------------------------------------------
def emulate_core(lay, pay, invdeg_rep, x_t, W):
    T, S = lay.T, lay.S
    relu = lambda v: np.maximum(v, 0.0)
    z = W["W1blk"].astype(np.float32).T @ pay.astype(np.float32)  # [112, T]
    r = relu(z)
    Ssum = np.zeros((112, S), np.float32)
    for (ci, ioff, oslot, nb, d) in lay.reduce_ops:
        seg = r[:, ci * CHUNK + ioff: ci * CHUNK + ioff + nb * d]
        Ssum[:, oslot:oslot + nb] = seg.reshape(112, nb, d).sum(axis=2)
    mean_r = Ssum * invdeg_rep                             # mean of relu'd
    # second edge-MLP layer pushed through the mean (linearity)
    mean = W["W1Bblk"].T @ mean_r + W["b1b_rep"]           # [112, S]
    sq = x_t * x_t                                         # [32, S]
    q = sq[8:16] + sq[16:24] + sq[24:32] - sq[0:8]         # [8, S]
    hn = relu(W["W2Ablk"].T @ mean + W["W2A0blk"].T @ q + W["b2a_rep"])
    out = W["W2Bblk"].T @ hn + W["b2b_rep"]                # [112, S]
    return out


# ---------------------------------------------------------------------------
# Bass device program
# ---------------------------------------------------------------------------
def build_bass_program(lay, dtype_np=np.float32):
    import concourse.bass as bass
    import concourse.bacc as bacc
    import concourse.tile as tile
    import concourse.mybir as mybir

    dt = mybir.dt.from_np(np.dtype(dtype_np))
    f32 = mybir.dt.float32
    T, S = lay.T, lay.S
    n_chunks = lay.n_chunks
    AF = mybir.ActivationFunctionType

    nc = bacc.Bacc("TRN2", target_bir_lowering=False, debug=False,
                   num_devices=NCORES)
    pay_d = nc.dram_tensor("payload", [128, T], dt, kind="ExternalInput")
    invdeg_d = nc.dram_tensor("invdeg", [112, S], f32, kind="ExternalInput")
    xt_d = nc.dram_tensor("x_t", [32, S], f32, kind="ExternalInput")
    w1_d = nc.dram_tensor("W1blk", [128, 112], dt, kind="ExternalInput")
    w1b_d = nc.dram_tensor("W1Bblk", [112, 112], f32, kind="ExternalInput")
    b1b_d = nc.dram_tensor("b1b_rep", [112, 1], f32, kind="ExternalInput")
    w2a_d = nc.dram_tensor("W2Ablk", [112, 112], f32, kind="ExternalInput")
    w2a0_d = nc.dram_tensor("W2A0blk", [8, 112], f32, kind="ExternalInput")
    b2a_d = nc.dram_tensor("b2a_rep", [112, 1], f32, kind="ExternalInput")
    w2b_d = nc.dram_tensor("W2Bblk", [112, 112], f32, kind="ExternalInput")
    b2b_d = nc.dram_tensor("b2b_rep", [112, 1], f32, kind="ExternalInput")
    out_d = nc.dram_tensor("out", [112, S], f32, kind="ExternalOutput")

    NSC = (S + MM_F - 1) // MM_F        # node chunks of 512 slots

    # split reduce ops at node-chunk (512-slot) output boundaries so each
    # Ssum sub-tile has an independent writer set -> node chunk k can start
    # as soon as its slots are reduced, overlapping the edge-phase tail
    ops_by_chunk = {}
    for (ci, ioff, oslot, nb, d) in lay.reduce_ops:
        while nb > 0:
            room = MM_F - (oslot % MM_F)
            take = min(nb, room)
            ops_by_chunk.setdefault(ci, []).append((ioff, oslot, take, d))
            ioff += take * d
            oslot += take
            nb -= take

    with tile.TileContext(nc) as tc:
        with (
            tc.tile_pool(name="const", bufs=1) as constp,
            tc.tile_pool(name="persist", bufs=1) as persist,
            tc.tile_pool(name="inp", bufs=3) as inp,
            tc.tile_pool(name="relu", bufs=8) as relup,
            tc.tile_pool(name="psum", bufs=2,
                         space=bass.MemorySpace.PSUM) as psum,
            tc.tile_pool(name="npsum", bufs=2,
                         space=bass.MemorySpace.PSUM) as npsum,
            tc.tile_pool(name="node", bufs=1) as nodep,
        ):
            cps = SUPER // CHUNK
            n_super = T // SUPER

            # W1 first (first matmul needs it), then the first super split
            # into per-chunk DMAs for fast pipeline ramp
            w1 = constp.tile([128, 112], dt)
            nc.sync.dma_start(w1[:], w1_d[:])
            pin0 = inp.tile([128, SUPER], dt, tag="pin")
            for cj in range(cps):
                nc.sync.dma_start(pin0[:, cj * CHUNK:(cj + 1) * CHUNK],
                                  pay_d[:, cj * CHUNK:(cj + 1) * CHUNK])

            # aux + node-phase constants (small; behind super 0 in the queue)
            invdeg = persist.tile([112, S], f32)
            nc.sync.dma_start(invdeg[:], invdeg_d[:])
            xk = []
            for k in range(4):
                t = persist.tile([8, S], f32, tag=f"xk{k}")
                nc.sync.dma_start(t[:], xt_d[8 * k:8 * (k + 1), :])
                xk.append(t)
            w1b = constp.tile([112, 112], f32)
            nc.sync.dma_start(w1b[:], w1b_d[:])
            b1b = constp.tile([112, 1], f32)
            nc.sync.dma_start(b1b[:], b1b_d[:])
            w2a = constp.tile([112, 112], f32)
            nc.sync.dma_start(w2a[:], w2a_d[:])
            w2a0 = constp.tile([8, 112], f32)
            nc.sync.dma_start(w2a0[:], w2a0_d[:])
            b2a = constp.tile([112, 1], f32)
            nc.sync.dma_start(b2a[:], b2a_d[:])
            w2b = constp.tile([112, 112], f32)
            nc.sync.dma_start(w2b[:], w2b_d[:])
            b2b = constp.tile([112, 1], f32)
            nc.sync.dma_start(b2b[:], b2b_d[:])

            # q = x1^2+x2^2+x3^2-x0^2, computed early: squares on ACT,
            # combines on the otherwise-idle GpSimd
            sqk = []
            for k in range(4):
                t = nodep.tile([8, S], f32, tag=f"sq{k}")
                nc.scalar.activation(t[:], xk[k][:], AF.Square)
                sqk.append(t)
            q8 = nodep.tile([8, S], f32)
            nc.gpsimd.tensor_add(q8[:], sqk[1][:], sqk[2][:])
            nc.gpsimd.tensor_add(q8[:], q8[:], sqk[3][:])
            nc.gpsimd.tensor_sub(q8[:], q8[:], sqk[0][:])

            Ssum = [nodep.tile([112, min(MM_F, S - k * MM_F)], f32,
                               name=f"Ssum{k}", tag=f"Ssum{k}")
                    for k in range(NSC)]

            # ---- edge phase: stream supers/chunks ----
            for si in range(n_super):
                if si == 0:
                    pin = pin0
                else:
                    pin = inp.tile([128, SUPER], dt, tag="pin")
                    nc.sync.dma_start(pin[:],
                                      pay_d[:, si * SUPER:(si + 1) * SUPER])
                for cj in range(cps):
                    ci = si * cps + cj
                    ps = psum.tile([112, CHUNK], f32, tag="ps")
                    for k in range(CHUNK // MM_F):
                        nc.tensor.matmul(
                            ps[:, k * MM_F:(k + 1) * MM_F],
                            w1[:],
                            pin[:, cj * CHUNK + k * MM_F:
                                cj * CHUNK + (k + 1) * MM_F],
                            start=True, stop=True,
                        )
                    rl = relup.tile([112, CHUNK], dt, tag="rl")
                    nc.scalar.activation(rl[:], ps[:], AF.Relu)
                    for (ioff, oslot, nb, d) in ops_by_chunk.get(ci, []):
                        seg = rl[:, ioff:ioff + nb * d]
                        seg3 = seg.rearrange("p (n d) -> p n d", d=d)
                        k = oslot // MM_F
                        lo = oslot - k * MM_F
                        nc.vector.reduce_sum(
                            out=Ssum[k][:, lo:lo + nb], in_=seg3,
                            axis=mybir.AxisListType.X)

            # ---- node phase (chunked; overlaps edge-phase tail) ----
            mean = nodep.tile([112, S], f32)
            hn = nodep.tile([112, S], f32)
            outt = nodep.tile([112, S], f32)
            for k in range(NSC):
                lo = k * MM_F
                hi = min(S, lo + MM_F)
                w = hi - lo
                mean_r = nodep.tile([112, w], f32, tag=f"mr{k}")
                nc.vector.tensor_mul(mean_r[:], Ssum[k][:],
                                     invdeg[:, lo:hi])
                # mean = W1Bblk^T @ mean_r + b1b
                p1 = npsum.tile([112, MM_F], f32, tag="np")
                nc.tensor.matmul(p1[:, :w], w1b[:], mean_r[:],
                                 start=True, stop=True)
                nc.scalar.activation(mean[:, lo:hi], p1[:, :w],
                                     AF.Identity, bias=b1b[:])
                # hn = relu(W2Ablk^T @ mean + W2A0blk^T @ q + b2a)
                p2 = npsum.tile([112, MM_F], f32, tag="np")
                nc.tensor.matmul(p2[:, :w], w2a[:], mean[:, lo:hi],
                                 start=True, stop=False)
                nc.tensor.matmul(p2[:, :w], w2a0[:], q8[:, lo:hi],
                                 start=False, stop=True)
                nc.scalar.activation(hn[:, lo:hi], p2[:, :w],
                                     AF.Relu, bias=b2a[:])
                # out = W2Bblk^T @ hn + b2b
                p3 = npsum.tile([112, MM_F], f32, tag="np")
                nc.tensor.matmul(p3[:, :w], w2b[:], hn[:, lo:hi],
                                 start=True, stop=True)
                nc.scalar.activation(outt[:, lo:hi], p3[:, :w],
                                     AF.Identity, bias=b2b[:])
                nc.sync.dma_start(out_d[:, lo:hi], outt[:, lo:hi])

    nc.compile()
    return nc


# ---------------------------------------------------------------------------
# kernel() entry point
# ---------------------------------------------------------------------------
def _prepare(x, edge_index, edge_attr, weights, dtype_np=np.float32):
    x = np.asarray(x, np.float32)
    edge_attr = np.asarray(edge_attr, np.float32)
    row = np.asarray(edge_index[0], np.int64)
    col = np.asarray(edge_index[1], np.int64)
    lay = build_layout(col)
    src_slots = build_src_slots(lay)
    q_nodes = ((x * _MINK) * x).sum(axis=1).astype(np.float32)
    qe = q_nodes[row]
    W = build_weights(*weights, dtype=dtype_np)
    per_core = []
    for c in range(NCORES):
        pay = build_core_payload(lay, src_slots, qe, edge_attr, c, dtype_np)
        invdeg_rep, x_t = build_core_aux(lay, x, c)
        per_core.append(dict(payload=pay, invdeg=invdeg_rep, x_t=x_t))
    return lay, W, per_core


def _assemble(lay, outs):
    """outs: list of [112, S] per core -> [N, 14]."""
    S = lay.S
    big = np.stack([o.reshape(NGROUPS, HID, S) for o in outs])  # [c, g, j, b]
    arr = big.transpose(3, 0, 1, 2).reshape(S * 64, HID)        # (b, c, g)
    res = np.empty((lay.deg.shape[0], HID), np.float32)
    res[lay.node_sorted] = arr
    return res[:N]


LAST_EXEC_TIME_NS = None
LAST_RESULTS = None


def kernel(x, edge_index, edge_attr, u, batch,
           w1a, b1a, w1b, b1b, w2a, b2a, w2b, b2b):
    global LAST_EXEC_TIME_NS, LAST_RESULTS
    _install_ntff_shim()
    weights = tuple(np.asarray(a, np.float32)
                    for a in (w1a, b1a, w1b, b1b, w2a, b2a, w2b, b2b))
    import ml_dtypes
    dtype_np = np.dtype(ml_dtypes.bfloat16)
    lay, W, per_core = _prepare(x, edge_index, edge_attr, weights, dtype_np)

    if os.environ.get("LNB_EMULATE"):
        outs = [emulate_core(lay, pc["payload"],
                             pc["invdeg"], pc["x_t"], W)
                for pc in per_core]
        return _assemble(lay, outs)

    from concourse.bass_utils import run_bass_kernel_spmd
    nc = build_bass_program(lay, dtype_np)
    in_maps = []
    for pc in per_core:
        in_maps.append({
            "payload": pc["payload"], "invdeg": pc["invdeg"],
            "x_t": pc["x_t"],
            "W1blk": W["W1blk"],
            "W1Bblk": W["W1Bblk"], "b1b_rep": W["b1b_rep"],
            "W2Ablk": W["W2Ablk"], "W2A0blk": W["W2A0blk"],
            "b2a_rep": W["b2a_rep"], "W2Bblk": W["W2Bblk"],
            "b2b_rep": W["b2b_rep"],
        })
    trace = bool(os.environ.get("BASS_TRACE"))
    res = run_bass_kernel_spmd(nc, in_maps, list(range(NCORES)), trace=trace)
    LAST_EXEC_TIME_NS = res.exec_time_ns
    LAST_RESULTS = res
    outs = [res.results[c]["out"] for c in range(NCORES)]
    return _assemble(lay, outs)


# revision 23
# speedup vs baseline: 1.1162x; 1.1162x over previous
"""LorentzNodeBlock — Trainium2 Bass kernel, 8 NeuronCores, scatter-free layout.

Sharding strategy (differs from the hint's edge-parallel+allreduce: we shard
by DESTINATION node so no collective is needed):

  * Host sorts nodes by in-degree and packs them into blocks of 64 nodes
    (one node per (core, group) lane; 8 cores x 8 groups = 64 lanes). Each
    node's incoming-edge list is zero-padded to the block's max degree, so
    all 64 lanes share one identical static layout -> one SPMD program.
  * Per-core edge payload is [120, T]: 8 groups x 15 rows (q_row, edge_attr^T)
    with edges along the free axis. q_row = minkowski(x)[row] is gathered on
    the host during layout construction (index-driven data staging); all
    model FLOPs on the [E,*] edge data happen on device.
  * Device per core: block-diagonal (8-way packed) first-MLP matmul on PE,
    fused bias+ReLU on the scalar engine, then segmented reduce_sum ops on
    the vector engine with static access patterns (uniform segment length
    per run of equal-degree blocks). The second edge-MLP linear layer is
    pushed through the segment sum (linearity): it is applied to the [N,14]
    node sums, not the [E,14] edge values. Zero-pad slots contribute
    relu(b1a) each; the device subtracts pad_count * relu(b1a) before the
    mean. Node MLP also runs on device in the same 8-group layout.
  * Host inverse-permutes the [112, S] per-core outputs back to [N, 14].
"""

import os
import sys
import types
import numpy as np

N = 100000
E = 6400000
HID = 14
NCORES = 8
NGROUPS = 8
NLANES = NCORES * NGROUPS
CHUNK = 1536          # edge slots per PSUM chunk (free dim, 3 banks)
SUPER = 6144          # edge slots per DMA transfer (4 chunks, 1.5 MB bf16)
MM_F = 512            # matmul moving free dim / PSUM bank (f32)

_MINK = np.array([-1.0, 1.0, 1.0, 1.0], dtype=np.float32)


# ---------------------------------------------------------------------------
# axon NTFF shim: lets run_bass_kernel_spmd(trace=True) work when the image's
# antenv package lacks axon_hooks. Harmless when tracing is off.
# ---------------------------------------------------------------------------
def _install_ntff_shim():
    try:
        from antenv.axon_hooks import get_axon_ntff_profile_hook  # noqa: F401
        have = True
    except ImportError:
        have = False
    if not have:
        try:
            import antenv
        except ImportError:
            return
        mod = types.ModuleType("antenv.axon_hooks")
        _hook = [None]
        mod.set_axon_ntff_profile_hook = lambda h: _hook.__setitem__(0, h)
        mod.get_axon_ntff_profile_hook = lambda: _hook[0]
        sys.modules["antenv.axon_hooks"] = mod
        antenv.axon_hooks = mod
    try:
        from antenv.axon_hooks import (get_axon_ntff_profile_hook,
                                       set_axon_ntff_profile_hook)
        if get_axon_ntff_profile_hook() is None:
            from trn_agent_boot.trn_boot import _ntff_profile_via_ctypes
            set_axon_ntff_profile_hook(
                _ntff_profile_via_ctypes('/opt/axon/libaxon_pjrt.so'))
    except Exception:
        pass


# ---------------------------------------------------------------------------
# Host-side layout
# ---------------------------------------------------------------------------
class Layout:
    __slots__ = ("node_sorted", "blk_deg", "blk_off", "n_blocks", "S", "T",
                 "n_chunks", "reduce_ops", "deg", "cs", "edge_sorted")


def build_layout(col):
    """col: int array [E] of destination nodes. Pure index work."""
    lay = Layout()
    deg = np.bincount(col, minlength=N).astype(np.int64)
    n_pad_nodes = (-N) % 64
    deg_p = np.concatenate([deg, np.zeros(n_pad_nodes, np.int64)])
    node_sorted = np.argsort(deg_p, kind="stable")
    B = deg_p.shape[0] // 64
    node_mat = node_sorted.reshape(B, 64)
    blk_deg = deg_p[node_mat].max(axis=1)          # [B] padded degree
    # pack blocks into CHUNK-aligned slots
    blk_off = np.zeros(B, np.int64)
    off = 0
    for b in range(B):
        d = int(blk_deg[b])
        if (off % CHUNK) + d > CHUNK:
            off = (off // CHUNK + 1) * CHUNK
        blk_off[b] = off
        off += d
    T = ((off + SUPER - 1) // SUPER) * SUPER
    # reduce ops: runs of equal-d blocks within one chunk
    # (chunk_idx, in_off_in_chunk, out_slot, n_blocks, d)
    ops = []
    b = 0
    while b < B:
        d = int(blk_deg[b])
        c0 = int(blk_off[b]) // CHUNK
        nb = 1
        while (b + nb < B and int(blk_deg[b + nb]) == d
               and int(blk_off[b + nb]) // CHUNK == c0
               and int(blk_off[b + nb]) == int(blk_off[b]) + nb * d):
            nb += 1
        if d > 0:
            ops.append((c0, int(blk_off[b]) % CHUNK, b, nb, d))
        b += nb
    cs = np.zeros(N + 1, np.int64)
    cs[1:] = np.cumsum(deg)
    lay.node_sorted = node_sorted
    lay.blk_deg = blk_deg
    lay.blk_off = blk_off
    lay.n_blocks = B
    lay.S = B
    lay.T = int(T)
    lay.n_chunks = int(T) // CHUNK
    lay.reduce_ops = ops
    lay.deg = deg_p
    lay.cs = cs
    lay.edge_sorted = np.argsort(col, kind="stable")
    return lay


def build_src_slots(lay):
    """[64, T] int64 edge-id per lane slot, -1 for padding."""
    B, T = lay.n_blocks, lay.T
    node_mat = lay.node_sorted.reshape(B, 64)
    deg_nm = lay.deg[node_mat]                       # [B, 64]
    D = int(lay.blk_deg.max())
    k = np.arange(D, dtype=np.int64)
    valid = k[None, None, :] < deg_nm[:, :, None]    # [B, 64, D]
    cs_nm = np.where(node_mat < N, lay.cs[np.minimum(node_mat, N - 1)], 0)
    idx = cs_nm[:, :, None] + k[None, None, :]
    idx = np.minimum(idx, E - 1)
    src = np.where(valid, lay.edge_sorted[idx], -1)  # [B, 64, D]
    slot_valid = k[None, :] < lay.blk_deg[:, None]   # [B, D]
    pos = (lay.blk_off[:, None] + k[None, :])        # [B, D]
    out = np.full((64, T), -1, np.int64)
    out[:, pos[slot_valid]] = src.transpose(1, 0, 2)[:, slot_valid]
    return out


def lane_of(c, g):
    return c * 8 + g


def build_core_payload(lay, src_slots, qe, edge_attr, core, dtype=np.float32):
    """[128, T]: rows 16g+0 = q_row, 16g+1+j = edge_attr[:, j],
    16g+15 = valid mask (1.0 real edge / 0.0 pad -> folds b1a into the
    matmul and makes pad slots contribute exactly 0 after relu)."""
    T = lay.T
    pay = np.zeros((NGROUPS * 16, T), dtype)
    for g in range(NGROUPS):
        s = src_slots[lane_of(core, g)]
        m = s >= 0
        sc = np.where(m, s, 0)
        pay[16 * g, :] = np.where(m, qe[sc], 0.0).astype(dtype)
        ea = edge_attr[sc].astype(dtype)
        ea[~m] = 0
        pay[16 * g + 1:16 * g + 15, :] = ea.T
        pay[16 * g + 15, :] = m.astype(dtype)
    return pay


def build_core_aux(lay, x, core):
    """invdeg_rep [112,S], x_t [32,S] (rows k*8+g)."""
    B = lay.n_blocks
    node_mat = lay.node_sorted.reshape(B, 64)
    nodes = node_mat[:, core * 8:(core + 1) * 8].T        # [8, B] (g, b)
    degs = lay.deg[nodes].astype(np.float32)              # [8, B]
    invdeg = 1.0 / np.maximum(degs, 1.0)
    invdeg_rep = np.repeat(invdeg, HID, axis=0)           # [112, B]
    real = nodes < N
    xn = x[np.minimum(nodes, N - 1)].astype(np.float32)   # [8, B, 4]
    xn[~real] = 0
    x_t = xn.transpose(2, 0, 1).reshape(32, B)            # rows k*8+g
    return invdeg_rep, x_t


def build_weights(w1a, b1a, w1b, b1b, w2a, b2a, w2b, b2b, dtype=np.float32):
    def blkdiag(w, nin, nout):
        out = np.zeros((NGROUPS * nin, NGROUPS * nout), np.float32)
        for g in range(NGROUPS):
            out[g * nin:(g + 1) * nin, g * nout:(g + 1) * nout] = w
        return out
    W = {}
    w1x = np.concatenate([w1a, b1a[None, :]], axis=0)      # [16, 14]
    W["W1blk"] = blkdiag(w1x, 16, 14).astype(dtype)        # [128, 112]
    W["b1a_rep"] = np.tile(b1a, NGROUPS).astype(np.float32)[:, None]  # [112,1]
    W["W1Bblk"] = blkdiag(w1b, 14, 14)                     # [112, 112]
    W["b1b_rep"] = np.tile(b1b, NGROUPS).astype(np.float32)[:, None]
    W["W2Ablk"] = blkdiag(w2a[1:], 14, 14)                 # [112, 112]
    w2a0 = np.zeros((NGROUPS, NGROUPS * 14), np.float32)
    for g in range(NGROUPS):
        w2a0[g, g * 14:(g + 1) * 14] = w2a[0]
    W["W2A0blk"] = w2a0                                    # [8, 112]
    W["b2a_rep"] = np.tile(b2a, NGROUPS).astype(np.float32)[:, None]
    W["W2Bblk"] = blkdiag(w2b, 14, 14)
    W["b2b_rep"] = np.tile(b2b, NGROUPS).astype(np.float32)[:, None]
    return W


# ---------------------------------------------------------------------------
# Numpy emulation of the device program (for layout/logic validation)
# ---------------------------------------------------------------------------
def emulate_core(lay, pay, invdeg_rep, x_t, W):
    T, S = lay.T, lay.S
    relu = lambda v: np.maximum(v, 0.0)
    z = W["W1blk"].astype(np.float32).T @ pay.astype(np.float32)  # [112, T]
    r = relu(z)
    Ssum = np.zeros((112, S), np.float32)
    for (ci, ioff, oslot, nb, d) in lay.reduce_ops:
        seg = r[:, ci * CHUNK + ioff: ci * CHUNK + ioff + nb * d]
        Ssum[:, oslot:oslot + nb] = seg.reshape(112, nb, d).sum(axis=2)
    mean_r = Ssum * invdeg_rep                             # mean of relu'd
    # second edge-MLP layer pushed through the mean (linearity)
    mean = W["W1Bblk"].T @ mean_r + W["b1b_rep"]           # [112, S]
    sq = x_t * x_t                                         # [32, S]
    q = sq[8:16] + sq[16:24] + sq[24:32] - sq[0:8]         # [8, S]
    hn = relu(W["W2Ablk"].T @ mean + W["W2A0blk"].T @ q + W["b2a_rep"])
    out = W["W2Bblk"].T @ hn + W["b2b_rep"]                # [112, S]
    return out


# ---------------------------------------------------------------------------
# Bass device program
# ---------------------------------------------------------------------------
def build_bass_program(lay, dtype_np=np.float32):
    import concourse.bass as bass
    import concourse.bacc as bacc
    import concourse.tile as tile
    import concourse.mybir as mybir

    dt = mybir.dt.from_np(np.dtype(dtype_np))
    f32 = mybir.dt.float32
    T, S = lay.T, lay.S
    n_chunks = lay.n_chunks
    AF = mybir.ActivationFunctionType

    nc = bacc.Bacc("TRN2", target_bir_lowering=False, debug=False,
                   num_devices=NCORES)
    pay_d = nc.dram_tensor("payload", [128, T], dt, kind="ExternalInput")
    invdeg_d = nc.dram_tensor("invdeg", [112, S], f32, kind="ExternalInput")
    xt_d = nc.dram_tensor("x_t", [32, S], f32, kind="ExternalInput")
    w1_d = nc.dram_tensor("W1blk", [128, 112], dt, kind="ExternalInput")
    w1b_d = nc.dram_tensor("W1Bblk", [112, 112], f32, kind="ExternalInput")
    b1b_d = nc.dram_tensor("b1b_rep", [112, 1], f32, kind="ExternalInput")
    w2a_d = nc.dram_tensor("W2Ablk", [112, 112], f32, kind="ExternalInput")
    w2a0_d = nc.dram_tensor("W2A0blk", [8, 112], f32, kind="ExternalInput")
    b2a_d = nc.dram_tensor("b2a_rep", [112, 1], f32, kind="ExternalInput")
    w2b_d = nc.dram_tensor("W2Bblk", [112, 112], f32, kind="ExternalInput")
    b2b_d = nc.dram_tensor("b2b_rep", [112, 1], f32, kind="ExternalInput")
    out_d = nc.dram_tensor("out", [112, S], f32, kind="ExternalOutput")

    NF = 256                            # node-chunk slot width
    NSC = (S + NF - 1) // NF

    # split reduce ops at node-chunk output boundaries so each Ssum
    # sub-tile has an independent writer set -> node chunk k can start
    # as soon as its slots are reduced, overlapping the edge-phase tail
    ops_by_chunk = {}
    nc_last_edge_chunk = [0] * NSC      # edge chunk that completes node chunk k
    for (ci, ioff, oslot, nb, d) in lay.reduce_ops:
        while nb > 0:
            room = NF - (oslot % NF)
            take = min(nb, room)
            ops_by_chunk.setdefault(ci, []).append((ioff, oslot, take, d))
            k = oslot // NF
            nc_last_edge_chunk[k] = max(nc_last_edge_chunk[k], ci)
            ioff += take * d
            oslot += take
            nb -= take

    with tile.TileContext(nc) as tc:
        with (
            tc.tile_pool(name="const", bufs=1) as constp,
            tc.tile_pool(name="persist", bufs=1) as persist,
            tc.tile_pool(name="inp", bufs=3) as inp,
            tc.tile_pool(name="relu", bufs=8) as relup,
            tc.tile_pool(name="psum", bufs=2,
                         space=bass.MemorySpace.PSUM) as psum,
            tc.tile_pool(name="npsum", bufs=2,
                         space=bass.MemorySpace.PSUM) as npsum,
            tc.tile_pool(name="node", bufs=1) as nodep,
        ):
            cps = SUPER // CHUNK
            n_super = T // SUPER

            # W1 first (first matmul needs it), then the first super split
            # into per-chunk DMAs for fast pipeline ramp
            w1 = constp.tile([128, 112], dt)
            nc.sync.dma_start(w1[:], w1_d[:])
            pin0 = inp.tile([128, SUPER], dt, tag="pin")
            for cj in range(cps):
                nc.sync.dma_start(pin0[:, cj * CHUNK:(cj + 1) * CHUNK],
                                  pay_d[:, cj * CHUNK:(cj + 1) * CHUNK])

            # aux + node-phase constants (small; behind super 0 in the queue)
            invdeg = persist.tile([112, S], f32)
            nc.sync.dma_start(invdeg[:], invdeg_d[:])
            xk = []
            for k in range(4):
                t = persist.tile([8, S], f32, tag=f"xk{k}")
                nc.sync.dma_start(t[:], xt_d[8 * k:8 * (k + 1), :])
                xk.append(t)
            w1b = constp.tile([112, 112], f32)
            nc.sync.dma_start(w1b[:], w1b_d[:])
            b1b = constp.tile([112, 1], f32)
            nc.sync.dma_start(b1b[:], b1b_d[:])
            w2a = constp.tile([112, 112], f32)
            nc.sync.dma_start(w2a[:], w2a_d[:])
            w2a0 = constp.tile([8, 112], f32)
            nc.sync.dma_start(w2a0[:], w2a0_d[:])
            b2a = constp.tile([112, 1], f32)
            nc.sync.dma_start(b2a[:], b2a_d[:])
            w2b = constp.tile([112, 112], f32)
            nc.sync.dma_start(w2b[:], w2b_d[:])
            b2b = constp.tile([112, 1], f32)
            nc.sync.dma_start(b2b[:], b2b_d[:])

            # q = x1^2+x2^2+x3^2-x0^2, entirely on the otherwise-idle GpSimd
            sqk = []
            for k in range(4):
                t = nodep.tile([8, S], f32, tag=f"sq{k}", name=f"sq{k}")
                nc.gpsimd.tensor_mul(t[:], xk[k][:], xk[k][:])
                sqk.append(t)
            q8 = nodep.tile([8, S], f32)
            nc.gpsimd.tensor_add(q8[:], sqk[1][:], sqk[2][:])
            nc.gpsimd.tensor_add(q8[:], q8[:], sqk[3][:])
            nc.gpsimd.tensor_sub(q8[:], q8[:], sqk[0][:])

            Ssum = [nodep.tile([112, min(NF, S - k * NF)], f32,
                               name=f"Ssum{k}", tag=f"Ssum{k}")
                    for k in range(NSC)]

            def emit_node_chunk(k):
                lo = k * NF
                hi = min(S, lo + NF)
                w = hi - lo
                mean_r = nodep.tile([112, w], f32, tag=f"mr{k}",
                                    name=f"mr{k}")
                nc.vector.tensor_mul(mean_r[:], Ssum[k][:], invdeg[:, lo:hi])
                # mean = W1Bblk^T @ mean_r + b1b
                p1 = npsum.tile([112, NF], f32, tag="np", name=f"np1_{k}")
                nc.tensor.matmul(p1[:, :w], w1b[:], mean_r[:],
                                 start=True, stop=True)
                mean = nodep.tile([112, w], f32, tag=f"mean{k}",
                                  name=f"mean{k}")
                nc.scalar.activation(mean[:], p1[:, :w],
                                     AF.Identity, bias=b1b[:])
                # hn = relu(W2Ablk^T @ mean + W2A0blk^T @ q + b2a)
                p2 = npsum.tile([112, NF], f32, tag="np", name=f"np2_{k}")
                nc.tensor.matmul(p2[:, :w], w2a[:], mean[:],
                                 start=True, stop=False)
                nc.tensor.matmul(p2[:, :w], w2a0[:], q8[:, lo:hi],
                                 start=False, stop=True)
                hn = nodep.tile([112, w], f32, tag=f"hn{k}", name=f"hn{k}")
                nc.scalar.activation(hn[:], p2[:, :w], AF.Relu, bias=b2a[:])
                # out = W2Bblk^T @ hn + b2b
                p3 = npsum.tile([112, NF], f32, tag="np", name=f"np3_{k}")
                nc.tensor.matmul(p3[:, :w], w2b[:], hn[:],
                                 start=True, stop=True)
                outt = nodep.tile([112, w], f32, tag=f"out{k}",
                                  name=f"out{k}")
                nc.scalar.activation(outt[:], p3[:, :w],
                                     AF.Identity, bias=b2b[:])
                # gpsimd (SWDGE) queue: keeps output stores out of the
                # payload stream's HWDGE FIFO
                nc.gpsimd.dma_start(out_d[:, lo:hi], outt[:])

            # node chunk k is emitted right after the edge chunk that
            # completes its Ssum slice (engines execute their streams in
            # order -> emission position controls overlap)
            node_after = {}
            for k in range(NSC):
                node_after.setdefault(nc_last_edge_chunk[k], []).append(k)

            # ---- edge phase: stream supers/chunks ----
            for si in range(n_super):
                if si == 0:
                    pin = pin0
                else:
                    pin = inp.tile([128, SUPER], dt, tag="pin")
                    nc.sync.dma_start(pin[:],
                                      pay_d[:, si * SUPER:(si + 1) * SUPER])
                for cj in range(cps):
                    ci = si * cps + cj
                    ps = psum.tile([112, CHUNK], f32, tag="ps")
                    for k in range(CHUNK // MM_F):
                        nc.tensor.matmul(
                            ps[:, k * MM_F:(k + 1) * MM_F],
                            w1[:],
                            pin[:, cj * CHUNK + k * MM_F:
                                cj * CHUNK + (k + 1) * MM_F],
                            start=True, stop=True,
                        )
                    rl = relup.tile([112, CHUNK], dt, tag="rl")
                    nc.scalar.activation(rl[:], ps[:], AF.Relu)
                    for (ioff, oslot, nb, d) in ops_by_chunk.get(ci, []):
                        seg = rl[:, ioff:ioff + nb * d]
                        seg3 = seg.rearrange("p (n d) -> p n d", d=d)
                        k = oslot // NF
                        lo = oslot - k * NF
                        nc.vector.reduce_sum(
                            out=Ssum[k][:, lo:lo + nb], in_=seg3,
                            axis=mybir.AxisListType.X)
                    for k in node_after.get(ci, []):
                        emit_node_chunk(k)

    nc.compile()
    return nc


# ---------------------------------------------------------------------------
# kernel() entry point
# ---------------------------------------------------------------------------
def _prepare(x, edge_index, edge_attr, weights, dtype_np=np.float32):
    x = np.asarray(x, np.float32)
    edge_attr = np.asarray(edge_attr, np.float32)
    row = np.asarray(edge_index[0], np.int64)
    col = np.asarray(edge_index[1], np.int64)
    lay = build_layout(col)
    src_slots = build_src_slots(lay)
    q_nodes = ((x * _MINK) * x).sum(axis=1).astype(np.float32)
    qe = q_nodes[row]
    W = build_weights(*weights, dtype=dtype_np)
    per_core = []
    for c in range(NCORES):
        pay = build_core_payload(lay, src_slots, qe, edge_attr, c, dtype_np)
        invdeg_rep, x_t = build_core_aux(lay, x, c)
        per_core.append(dict(payload=pay, invdeg=invdeg_rep, x_t=x_t))
    return lay, W, per_core


def _assemble(lay, outs):
    """outs: list of [112, S] per core -> [N, 14]."""
    S = lay.S
    big = np.stack([o.reshape(NGROUPS, HID, S) for o in outs])  # [c, g, j, b]
    arr = big.transpose(3, 0, 1, 2).reshape(S * 64, HID)        # (b, c, g)
    res = np.empty((lay.deg.shape[0], HID), np.float32)
    res[lay.node_sorted] = arr
    return res[:N]


LAST_EXEC_TIME_NS = None
LAST_RESULTS = None


def kernel(x, edge_index, edge_attr, u, batch,
           w1a, b1a, w1b, b1b, w2a, b2a, w2b, b2b):
    global LAST_EXEC_TIME_NS, LAST_RESULTS
    _install_ntff_shim()
    weights = tuple(np.asarray(a, np.float32)
                    for a in (w1a, b1a, w1b, b1b, w2a, b2a, w2b, b2b))
    import ml_dtypes
    dtype_np = np.dtype(ml_dtypes.bfloat16)
    lay, W, per_core = _prepare(x, edge_index, edge_attr, weights, dtype_np)

    if os.environ.get("LNB_EMULATE"):
        outs = [emulate_core(lay, pc["payload"],
                             pc["invdeg"], pc["x_t"], W)
                for pc in per_core]
        return _assemble(lay, outs)

    from concourse.bass_utils import run_bass_kernel_spmd
    nc = build_bass_program(lay, dtype_np)
    in_maps = []
    for pc in per_core:
        in_maps.append({
            "payload": pc["payload"], "invdeg": pc["invdeg"],
            "x_t": pc["x_t"],
            "W1blk": W["W1blk"],
            "W1Bblk": W["W1Bblk"], "b1b_rep": W["b1b_rep"],
            "W2Ablk": W["W2Ablk"], "W2A0blk": W["W2A0blk"],
            "b2a_rep": W["b2a_rep"], "W2Bblk": W["W2Bblk"],
            "b2b_rep": W["b2b_rep"],
        })
    trace = bool(os.environ.get("BASS_TRACE"))
    res = run_bass_kernel_spmd(nc, in_maps, list(range(NCORES)), trace=trace)
    LAST_EXEC_TIME_NS = res.exec_time_ns
    LAST_RESULTS = res
    outs = [res.results[c]["out"] for c in range(NCORES)]
    return _assemble(lay, outs)
